# revision 29
# baseline (speedup 1.0000x reference)
"""Causal MHA + RoPE on 8 TRN2 NeuronCores — v4 (see kernel.py docstring)."""
import math
import numpy as np
import ml_dtypes

import concourse.bass as bass
import concourse.mybir as mybir
import concourse.tile as tile
from concourse import bacc
from concourse.bass import ds
from concourse.bass_utils import run_bass_kernel_spmd

F32 = mybir.dt.float32
BF16 = mybir.dt.bfloat16
EXP = mybir.ActivationFunctionType.Exp

D_MODEL = 1024
DK = 64
THETA = 10000.0
B, S = 2, 2048
HPC = 4
EL = HPC * DK
SCALE = 1.0 / math.sqrt(DK)
NQ = 512
NT = 128
NSPAN = S // NQ
NKT = S // NT
DCH = D_MODEL // 128
VW = DK + 1

_CACHE = {}


def _build_nc():
    nc = bacc.Bacc(None, target_bir_lowering=False)
    xT = nc.declare_dram_parameter("xT", [D_MODEL, S], BF16, isOutput=False)
    wq = nc.declare_dram_parameter("wq", [D_MODEL, EL], BF16, isOutput=False)
    wk = nc.declare_dram_parameter("wk", [D_MODEL, EL], BF16, isOutput=False)
    wv = nc.declare_dram_parameter("wv", [D_MODEL, EL], BF16, isOutput=False)
    wo = nc.declare_dram_parameter("wo", [EL, D_MODEL], BF16, isOutput=False)
    cosT = nc.declare_dram_parameter("cosT", [128, S], BF16, isOutput=False)
    sinT = nc.declare_dram_parameter("sinT", [128, S], BF16, isOutput=False)
    y = nc.declare_dram_parameter("y", [S, D_MODEL], F32, isOutput=True)

    with tile.TileContext(nc) as tc:
        with (
            tc.tile_pool(name="p_fin", bufs=1) as p_fin,
            tc.tile_pool(name="p_work", bufs=1) as p_work,
            tc.tile_pool(name="ps", bufs=1, space="PSUM") as ps,
        ):
            # ---- persistent tiles ----
            qt_fin = p_fin.tile([128, 2 * S], BF16, tag="qt_fin", name="qt_fin")
            kt_fin = p_fin.tile([128, 2 * S], BF16, tag="kt_fin", name="kt_fin")
            v_aug = p_fin.tile([128, NKT * HPC * VW], BF16, tag="v_aug", name="v_aug")
            ones4 = p_fin.tile([128, HPC], F32, tag="ones4", name="ones4")
            nc.vector.memset(ones4, 1.0)
            attnT = [p_fin.tile([128, S], BF16, tag=f"attnT{p}", name=f"attnT{p}")
                     for p in range(2)]

            # ---- input DMAs, spread across queues so nothing serializes ----
            wq_sb = p_fin.tile([128, DCH * EL], BF16, tag="wq", name="wq_sb")
            wk_sb = p_fin.tile([128, DCH * EL], BF16, tag="wk", name="wk_sb")
            wv_sb = p_fin.tile([128, DCH * EL], BF16, tag="wv", name="wv_sb")
            nc.scalar.dma_start(out=wq_sb.rearrange("p (d e) -> p d e", d=DCH),
                                in_=wq.rearrange("(d p) e -> p d e", p=128))
            nc.gpsimd.dma_start(out=wk_sb.rearrange("p (d e) -> p d e", d=DCH),
                                in_=wk.rearrange("(d p) e -> p d e", p=128))
            nc.gpsimd.dma_start(out=wv_sb.rearrange("p (d e) -> p d e", d=DCH),
                                in_=wv.rearrange("(d p) e -> p d e", p=128))
            cos_sb = p_fin.tile([128, S], BF16, tag="cos", name="cos_sb")
            sin_sb = p_fin.tile([128, S], BF16, tag="sin", name="sin_sb")
            nc.scalar.dma_start(out=cos_sb, in_=cosT[:, :])
            nc.gpsimd.dma_start(out=sin_sb, in_=sinT[:, :])
            wo_sb = p_fin.tile([128, 2 * D_MODEL], BF16, tag="wo", name="wo_sb")
            nc.gpsimd.dma_start(out=wo_sb.rearrange("p (c e) -> p c e", c=2),
                                in_=wo.rearrange("(c p) e -> p c e", p=128))

            # ---- phase emitters ----
            def emit_b(j2):
                xp = [p_fin.tile([128, 1024], BF16, tag="xt", bufs=16, name="xt")
                      for _ in range(DCH)]
                for h5 in range(2):
                    for d in range(DCH):
                        nc.sync.dma_start(
                            out=xp[d][:, ds(h5 * NQ, NQ)],
                            in_=xT[d * 128:(d + 1) * 128,
                                   j2 * 1024 + h5 * NQ:j2 * 1024 + (h5 + 1) * NQ])
                for kind, w_sb in ((0, wq_sb), (1, wk_sb)):
                    for c in range(2):
                        pq = ps.tile([128, 1024], F32, tag="big", bufs=2, name="pq")
                        for h5 in range(2):
                            for d in range(DCH):
                                nc.tensor.matmul(
                                    pq[:, ds(h5 * NQ, NQ)],
                                    w_sb[:, ds(d * EL + c * 128, 128)],
                                    xp[d][:, ds(h5 * NQ, NQ)],
                                    start=(d == 0), stop=(d == DCH - 1))
                        raw = p_work.tile([128, 1024], BF16, tag="raw", bufs=4, name="raw")
                        nc.scalar.copy(raw, pq)
                        swp = p_work.tile([128, 1024], BF16, tag="swp", bufs=4, name="swp")
                        for blk in range(2):
                            b0 = blk * 64
                            nc.vector.tensor_copy(swp[b0:b0 + 32, :], raw[b0 + 32:b0 + 64, :])
                            nc.vector.tensor_copy(swp[b0 + 32:b0 + 64, :], raw[b0:b0 + 32, :])
                        sl2 = ds(j2 * 1024, 1024)
                        fin = qt_fin if kind == 0 else kt_fin
                        nc.vector.tensor_mul(raw, raw, cos_sb[:, sl2])
                        nc.vector.tensor_mul(swp, swp, sin_sb[:, sl2])
                        nc.vector.tensor_add(fin[:, ds(c * S + j2 * 1024, 1024)], raw, swp)
                for sti in range(8):
                    t = 8 * j2 + sti
                    pv = ps.tile([128, 1024], F32, tag="big", bufs=2, name="pv")
                    for d in range(DCH):
                        nc.tensor.matmul(
                            pv[:, 0:EL],
                            xp[d][:, ds(sti * 128, 128)],
                            wv_sb[:, ds(d * EL, EL)],
                            start=(d == 0), stop=(d == DCH - 1))
                    vview = v_aug[:, ds(t * HPC * VW, HPC * VW)].rearrange(
                        "p (h a) -> p h a", a=VW)
                    nc.scalar.copy(vview[:, :, 0:DK],
                                   pv[:, 0:EL].rearrange("p (h m) -> p h m", m=DK))
                    nc.vector.tensor_copy(vview[:, :, DK:VW],
                                          ones4.rearrange("p (h o) -> p h o", o=1))

            def emit_attn(p, j):
                pva = ps.tile([128, NQ], F32, tag="acc", bufs=4, name="pva")
                pvb = ps.tile([128, NQ], F32, tag="acc", bufs=4, name="pvb")
                ntile = 4 * j + 4
                for t in range(ntile):
                    dd = max(0, t - 4 * j)          # diagonal offset 0..3
                    q0 = dd * NT                    # first live q-col in this tile
                    w = NQ - q0                     # live width per head half
                    stp = ps.tile([128, 2 * NQ], F32, tag="big", bufs=2, name="stp")
                    for hh in range(2):
                        nc.tensor.matmul(
                            stp[:, ds(hh * NQ + q0, w)],
                            kt_fin[hh * 64:(hh + 1) * 64, ds(p * S + t * NT, NT)],
                            qt_fin[hh * 64:(hh + 1) * 64, ds(p * S + j * NQ + q0, w)],
                            start=True, stop=True)
                    ste = p_work.tile([128, 2 * NQ], BF16, tag="ste", bufs=6, name="ste")
                    wf = 2 * NQ - q0   # one flat segment; dead middle cols unread
                    nc.scalar.activation(ste[:, ds(q0, wf)], stp[:, ds(q0, wf)],
                                         EXP, scale=SCALE)
                    if t >= 4 * j:
                        nc.vector.tensor_mul(ste[:, ds(q0, wf)], ste[:, ds(q0, wf)],
                                             mask4[dd][:, ds(q0, wf)])
                    for hh, pvx in ((0, pva), (1, pvb)):
                        nc.tensor.matmul(
                            pvx[0:VW, ds(q0, w)],
                            v_aug[:, ds(t * HPC * VW + (2 * p + hh) * VW, VW)],
                            ste[:, ds(hh * NQ + q0, w)],
                            start=(t == 0), stop=(t == ntile - 1))
                lcp_a = p_work.tile([1, NQ], F32, tag="lcp_a", bufs=3, name="lcp_a")
                lcp_b = p_work.tile([1, NQ], F32, tag="lcp_b", bufs=3, name="lcp_b")
                nc.vector.tensor_copy(lcp_a, pva[64:65, :])
                nc.vector.tensor_copy(lcp_b, pvb[64:65, :])
                recl_a = p_work.tile([1, NQ], F32, tag="recl_a", bufs=3, name="recl_a")
                recl_b = p_work.tile([1, NQ], F32, tag="recl_b", bufs=3, name="recl_b")
                nc.vector.reciprocal_approx_fast(recl_a, lcp_a)
                nc.vector.reciprocal_approx_fast(recl_b, lcp_b)
                rb_a = p_work.tile([64, NQ], F32, tag="rb_a", bufs=3, name="rb_a")
                rb_b = p_work.tile([64, NQ], F32, tag="rb_b", bufs=3, name="rb_b")
                nc.gpsimd.partition_broadcast(rb_a, recl_a, channels=64)
                nc.gpsimd.partition_broadcast(rb_b, recl_b, channels=64)
                sl = ds(j * NQ, NQ)
                nc.vector.tensor_mul(attnT[p][0:64, sl], pva[0:64, :], rb_a)
                nc.vector.tensor_mul(attnT[p][64:128, sl], pvb[0:64, :], rb_b)

            def emit_e(j):
                for sti in range(4 * j, 4 * j + 4):
                    for e2 in range(2):
                        py = ps.tile([128, NQ], F32, tag="acc", bufs=4, name="py")
                        for c in range(2):
                            nc.tensor.matmul(
                                py[:, 0:NQ],
                                attnT[c][:, ds(sti * 128, 128)],
                                wo_sb[:, ds(c * D_MODEL + e2 * NQ, NQ)],
                                start=(c == 0), stop=(c == 1))
                        ysb = p_work.tile([128, NQ], F32, tag="ysb", bufs=6, name="ysb")
                        nc.vector.tensor_copy(ysb, py[:, 0:NQ])
                        eng = nc.sync if e2 == 0 else nc.gpsimd
                        eng.dma_start(
                            out=y[sti * 128:(sti + 1) * 128, e2 * NQ:(e2 + 1) * NQ],
                            in_=ysb)

            # PE warmup during initial DMA wait: matmuls on mask data
            warm_ps = ps.tile([128, NQ], F32, tag="acc", bufs=4, name="warm_ps")
            emit_b(0)
            # causal 0/1 masks for the 4 diagonal offsets, same pattern per head half
            mask4 = []
            for dd in range(4):
                m = p_fin.tile([128, 2 * NQ], BF16, tag=f"mask{dd}", name=f"mask{dd}")
                nc.vector.memset(m, 1.0)
                nc.gpsimd.affine_select(
                    out=m.rearrange("p (h q) -> p h q", h=2),
                    in_=m.rearrange("p (h q) -> p h q", h=2),
                    compare_op=mybir.AluOpType.is_ge,
                    fill=0.0, base=-NT * dd,
                    pattern=[[0, 2], [1, NQ]],
                    channel_multiplier=-1,
                )
                mask4.append(m)
            emit_attn(0, 0)
            emit_b(1)
            emit_attn(1, 0)
            emit_attn(0, 1)
            emit_e(0)
            emit_attn(1, 1)
            emit_attn(0, 2)
            emit_e(1)
            emit_attn(1, 2)
            emit_attn(0, 3)
            emit_e(2)
            emit_attn(1, 3)
            emit_e(3)
    nc.finalize()
    return nc


def _host_prep(x, Wq, Wk, Wv, Wo):
    x = np.asarray(x, dtype=np.float32)
    Wq, Wk, Wv, Wo = (np.asarray(w, dtype=np.float32) for w in (Wq, Wk, Wv, Wo))
    bf = ml_dtypes.bfloat16

    p64 = np.concatenate([np.arange(0, DK, 2), np.arange(1, DK, 2)])
    freqs = 1.0 / THETA ** (np.arange(0, DK, 2, dtype=np.float64) / DK)
    ang = np.arange(S, dtype=np.float64)[None, :] * freqs[:, None]
    cos32 = np.cos(ang).astype(np.float32)
    sin32 = np.sin(ang).astype(np.float32)
    cosT = np.ascontiguousarray(np.tile(cos32, (4, 1))).astype(bf)
    sinT = np.ascontiguousarray(
        np.concatenate([-sin32, sin32, -sin32, sin32], axis=0)).astype(bf)

    xTs = [np.ascontiguousarray(x[b].T).astype(bf) for b in range(B)]
    perm = np.concatenate([h * DK + p64 for h in range(HPC)])

    in_maps = []
    for core in range(8):
        bg, hg = core // 4, core % 4
        sl = slice(hg * EL, (hg + 1) * EL)
        in_maps.append({
            "xT": xTs[bg],
            "wq": np.ascontiguousarray(Wq[sl][perm].T).astype(bf),
            "wk": np.ascontiguousarray(Wk[sl][perm].T).astype(bf),
            "wv": np.ascontiguousarray(Wv[sl].T).astype(bf),
            "wo": np.ascontiguousarray(Wo[:, sl].T).astype(bf),
            "cosT": cosT,
            "sinT": sinT,
        })
    return in_maps


def kernel(x, Wq, Wk, Wv, Wo, _trace=False):
    if "nc" not in _CACHE:
        _CACHE["nc"] = _build_nc()
    nc = _CACHE["nc"]
    in_maps = _host_prep(x, Wq, Wk, Wv, Wo)
    res = run_bass_kernel_spmd(nc, in_maps, core_ids=list(range(8)), trace=_trace)
    _CACHE["last_result"] = res
    out = np.zeros((B, S, D_MODEL), dtype=np.float32)
    for core in range(8):
        out[core // 4] += res.results[core]["y"]
    return out


# revision 30
# speedup vs baseline: 1.0071x; 1.0071x over previous
"""Causal MHA + RoPE on 8 TRN2 NeuronCores — v4 (see kernel.py docstring)."""
import math
import numpy as np
import ml_dtypes

import concourse.bass as bass
import concourse.mybir as mybir
import concourse.tile as tile
from concourse import bacc
from concourse.bass import ds
from concourse.bass_utils import run_bass_kernel_spmd

F32 = mybir.dt.float32
BF16 = mybir.dt.bfloat16
EXP = mybir.ActivationFunctionType.Exp

D_MODEL = 1024
DK = 64
THETA = 10000.0
B, S = 2, 2048
HPC = 4
EL = HPC * DK
SCALE = 1.0 / math.sqrt(DK)
NQ = 512
NT = 128
NSPAN = S // NQ
NKT = S // NT
DCH = D_MODEL // 128
VW = DK + 1

_CACHE = {}


def _build_nc():
    nc = bacc.Bacc(None, target_bir_lowering=False)
    xT = nc.declare_dram_parameter("xT", [D_MODEL, S], BF16, isOutput=False)
    wq = nc.declare_dram_parameter("wq", [D_MODEL, EL], BF16, isOutput=False)
    wk = nc.declare_dram_parameter("wk", [D_MODEL, EL], BF16, isOutput=False)
    wv = nc.declare_dram_parameter("wv", [D_MODEL, EL], BF16, isOutput=False)
    wo = nc.declare_dram_parameter("wo", [EL, D_MODEL], BF16, isOutput=False)
    cosT = nc.declare_dram_parameter("cosT", [128, S], BF16, isOutput=False)
    sinT = nc.declare_dram_parameter("sinT", [128, S], BF16, isOutput=False)
    y = nc.declare_dram_parameter("y", [S, D_MODEL], F32, isOutput=True)

    with tile.TileContext(nc) as tc:
        with (
            tc.tile_pool(name="p_fin", bufs=1) as p_fin,
            tc.tile_pool(name="p_work", bufs=1) as p_work,
            tc.tile_pool(name="ps", bufs=1, space="PSUM") as ps,
        ):
            # ---- persistent tiles ----
            qt_fin = p_fin.tile([128, 2 * S], BF16, tag="qt_fin", name="qt_fin")
            kt_fin = p_fin.tile([128, 2 * S], BF16, tag="kt_fin", name="kt_fin")
            v_aug = p_fin.tile([128, NKT * HPC * VW], BF16, tag="v_aug", name="v_aug")
            ones4 = p_fin.tile([128, HPC], F32, tag="ones4", name="ones4")
            nc.vector.memset(ones4, 1.0)
            attnT = [p_fin.tile([128, S], BF16, tag=f"attnT{p}", name=f"attnT{p}")
                     for p in range(2)]

            # ---- input DMAs, spread across queues so nothing serializes ----
            wq_sb = p_fin.tile([128, DCH * EL], BF16, tag="wq", name="wq_sb")
            wk_sb = p_fin.tile([128, DCH * EL], BF16, tag="wk", name="wk_sb")
            wv_sb = p_fin.tile([128, DCH * EL], BF16, tag="wv", name="wv_sb")
            nc.scalar.dma_start(out=wq_sb.rearrange("p (d e) -> p d e", d=DCH),
                                in_=wq.rearrange("(d p) e -> p d e", p=128))
            nc.gpsimd.dma_start(out=wk_sb.rearrange("p (d e) -> p d e", d=DCH),
                                in_=wk.rearrange("(d p) e -> p d e", p=128))
            nc.gpsimd.dma_start(out=wv_sb.rearrange("p (d e) -> p d e", d=DCH),
                                in_=wv.rearrange("(d p) e -> p d e", p=128))
            cos_sb = p_fin.tile([128, S], BF16, tag="cos", name="cos_sb")
            sin_sb = p_fin.tile([128, S], BF16, tag="sin", name="sin_sb")
            nc.scalar.dma_start(out=cos_sb, in_=cosT[:, :])
            nc.gpsimd.dma_start(out=sin_sb, in_=sinT[:, :])
            wo_sb = p_fin.tile([128, 2 * D_MODEL], BF16, tag="wo", name="wo_sb")
            nc.gpsimd.dma_start(out=wo_sb.rearrange("p (c e) -> p c e", c=2),
                                in_=wo.rearrange("(c p) e -> p c e", p=128))

            # ---- phase emitters ----
            def emit_b(j2):
                xp = [p_fin.tile([128, 1024], BF16, tag="xt", bufs=16, name="xt")
                      for _ in range(DCH)]
                for h5 in range(2):
                    for d in range(DCH):
                        nc.sync.dma_start(
                            out=xp[d][:, ds(h5 * NQ, NQ)],
                            in_=xT[d * 128:(d + 1) * 128,
                                   j2 * 1024 + h5 * NQ:j2 * 1024 + (h5 + 1) * NQ])
                for kind, w_sb in ((0, wq_sb), (1, wk_sb)):
                    for c in range(2):
                        pq = ps.tile([128, 1024], F32, tag="big", bufs=2, name="pq")
                        for h5 in range(2):
                            for d in range(DCH):
                                nc.tensor.matmul(
                                    pq[:, ds(h5 * NQ, NQ)],
                                    w_sb[:, ds(d * EL + c * 128, 128)],
                                    xp[d][:, ds(h5 * NQ, NQ)],
                                    start=(d == 0), stop=(d == DCH - 1))
                        raw = p_work.tile([128, 1024], BF16, tag="raw", bufs=6, name="raw")
                        nc.scalar.copy(raw, pq)
                        swp = p_work.tile([128, 1024], BF16, tag="swp", bufs=6, name="swp")
                        for blk in range(2):
                            b0 = blk * 64
                            nc.vector.tensor_copy(swp[b0:b0 + 32, :], raw[b0 + 32:b0 + 64, :])
                            nc.vector.tensor_copy(swp[b0 + 32:b0 + 64, :], raw[b0:b0 + 32, :])
                        sl2 = ds(j2 * 1024, 1024)
                        fin = qt_fin if kind == 0 else kt_fin
                        nc.vector.tensor_mul(raw, raw, cos_sb[:, sl2])
                        nc.vector.tensor_mul(swp, swp, sin_sb[:, sl2])
                        nc.vector.tensor_add(fin[:, ds(c * S + j2 * 1024, 1024)], raw, swp)
                for sti in range(8):
                    t = 8 * j2 + sti
                    pv = ps.tile([128, 1024], F32, tag="big", bufs=2, name="pv")
                    for d in range(DCH):
                        nc.tensor.matmul(
                            pv[:, 0:EL],
                            xp[d][:, ds(sti * 128, 128)],
                            wv_sb[:, ds(d * EL, EL)],
                            start=(d == 0), stop=(d == DCH - 1))
                    vview = v_aug[:, ds(t * HPC * VW, HPC * VW)].rearrange(
                        "p (h a) -> p h a", a=VW)
                    nc.scalar.copy(vview[:, :, 0:DK],
                                   pv[:, 0:EL].rearrange("p (h m) -> p h m", m=DK))
                    nc.vector.tensor_copy(vview[:, :, DK:VW],
                                          ones4.rearrange("p (h o) -> p h o", o=1))

            def emit_attn(p, j):
                pva = ps.tile([128, NQ], F32, tag="acc", bufs=4, name="pva")
                pvb = ps.tile([128, NQ], F32, tag="acc", bufs=4, name="pvb")
                ntile = 4 * j + 4
                for t in range(ntile):
                    dd = max(0, t - 4 * j)          # diagonal offset 0..3
                    q0 = dd * NT                    # first live q-col in this tile
                    w = NQ - q0                     # live width per head half
                    stp = ps.tile([128, 2 * NQ], F32, tag="big", bufs=2, name="stp")
                    for hh in range(2):
                        nc.tensor.matmul(
                            stp[:, ds(hh * NQ + q0, w)],
                            kt_fin[hh * 64:(hh + 1) * 64, ds(p * S + t * NT, NT)],
                            qt_fin[hh * 64:(hh + 1) * 64, ds(p * S + j * NQ + q0, w)],
                            start=True, stop=True)
                    ste = p_work.tile([128, 2 * NQ], BF16, tag="ste", bufs=8, name="ste")
                    wf = 2 * NQ - q0   # one flat segment; dead middle cols unread
                    nc.scalar.activation(ste[:, ds(q0, wf)], stp[:, ds(q0, wf)],
                                         EXP, scale=SCALE)
                    if t >= 4 * j:
                        nc.vector.tensor_mul(ste[:, ds(q0, wf)], ste[:, ds(q0, wf)],
                                             mask4[dd][:, ds(q0, wf)])
                    for hh, pvx in ((0, pva), (1, pvb)):
                        nc.tensor.matmul(
                            pvx[0:VW, ds(q0, w)],
                            v_aug[:, ds(t * HPC * VW + (2 * p + hh) * VW, VW)],
                            ste[:, ds(hh * NQ + q0, w)],
                            start=(t == 0), stop=(t == ntile - 1))
                lcp_a = p_work.tile([1, NQ], F32, tag="lcp_a", bufs=3, name="lcp_a")
                lcp_b = p_work.tile([1, NQ], F32, tag="lcp_b", bufs=3, name="lcp_b")
                nc.vector.tensor_copy(lcp_a, pva[64:65, :])
                nc.vector.tensor_copy(lcp_b, pvb[64:65, :])
                recl_a = p_work.tile([1, NQ], F32, tag="recl_a", bufs=3, name="recl_a")
                recl_b = p_work.tile([1, NQ], F32, tag="recl_b", bufs=3, name="recl_b")
                nc.vector.reciprocal_approx_fast(recl_a, lcp_a)
                nc.vector.reciprocal_approx_fast(recl_b, lcp_b)
                rb_a = p_work.tile([64, NQ], F32, tag="rb_a", bufs=3, name="rb_a")
                rb_b = p_work.tile([64, NQ], F32, tag="rb_b", bufs=3, name="rb_b")
                nc.gpsimd.partition_broadcast(rb_a, recl_a, channels=64)
                nc.gpsimd.partition_broadcast(rb_b, recl_b, channels=64)
                sl = ds(j * NQ, NQ)
                nc.vector.tensor_mul(attnT[p][0:64, sl], pva[0:64, :], rb_a)
                nc.vector.tensor_mul(attnT[p][64:128, sl], pvb[0:64, :], rb_b)

            def emit_e(j):
                for sti in range(4 * j, 4 * j + 4):
                    for e2 in range(2):
                        py = ps.tile([128, NQ], F32, tag="acc", bufs=4, name="py")
                        for c in range(2):
                            nc.tensor.matmul(
                                py[:, 0:NQ],
                                attnT[c][:, ds(sti * 128, 128)],
                                wo_sb[:, ds(c * D_MODEL + e2 * NQ, NQ)],
                                start=(c == 0), stop=(c == 1))
                        ysb = p_work.tile([128, NQ], F32, tag="ysb", bufs=8, name="ysb")
                        nc.vector.tensor_copy(ysb, py[:, 0:NQ])
                        eng = nc.sync if e2 == 0 else nc.gpsimd
                        eng.dma_start(
                            out=y[sti * 128:(sti + 1) * 128, e2 * NQ:(e2 + 1) * NQ],
                            in_=ysb)

            # PE warmup during initial DMA wait: matmuls on mask data
            warm_ps = ps.tile([128, NQ], F32, tag="acc", bufs=4, name="warm_ps")
            emit_b(0)
            # causal 0/1 masks for the 4 diagonal offsets, same pattern per head half
            mask4 = []
            for dd in range(4):
                m = p_fin.tile([128, 2 * NQ], BF16, tag=f"mask{dd}", name=f"mask{dd}")
                nc.vector.memset(m, 1.0)
                nc.gpsimd.affine_select(
                    out=m.rearrange("p (h q) -> p h q", h=2),
                    in_=m.rearrange("p (h q) -> p h q", h=2),
                    compare_op=mybir.AluOpType.is_ge,
                    fill=0.0, base=-NT * dd,
                    pattern=[[0, 2], [1, NQ]],
                    channel_multiplier=-1,
                )
                mask4.append(m)
            emit_attn(0, 0)
            emit_b(1)
            emit_attn(1, 0)
            emit_attn(0, 1)
            emit_e(0)
            emit_attn(1, 1)
            emit_attn(0, 2)
            emit_e(1)
            emit_attn(1, 2)
            emit_attn(0, 3)
            emit_e(2)
            emit_attn(1, 3)
            emit_e(3)
    nc.finalize()
    return nc


def _host_prep(x, Wq, Wk, Wv, Wo):
    x = np.asarray(x, dtype=np.float32)
    Wq, Wk, Wv, Wo = (np.asarray(w, dtype=np.float32) for w in (Wq, Wk, Wv, Wo))
    bf = ml_dtypes.bfloat16

    p64 = np.concatenate([np.arange(0, DK, 2), np.arange(1, DK, 2)])
    freqs = 1.0 / THETA ** (np.arange(0, DK, 2, dtype=np.float64) / DK)
    ang = np.arange(S, dtype=np.float64)[None, :] * freqs[:, None]
    cos32 = np.cos(ang).astype(np.float32)
    sin32 = np.sin(ang).astype(np.float32)
    cosT = np.ascontiguousarray(np.tile(cos32, (4, 1))).astype(bf)
    sinT = np.ascontiguousarray(
        np.concatenate([-sin32, sin32, -sin32, sin32], axis=0)).astype(bf)

    xTs = [np.ascontiguousarray(x[b].T).astype(bf) for b in range(B)]
    perm = np.concatenate([h * DK + p64 for h in range(HPC)])

    in_maps = []
    for core in range(8):
        bg, hg = core // 4, core % 4
        sl = slice(hg * EL, (hg + 1) * EL)
        in_maps.append({
            "xT": xTs[bg],
            "wq": np.ascontiguousarray(Wq[sl][perm].T).astype(bf),
            "wk": np.ascontiguousarray(Wk[sl][perm].T).astype(bf),
            "wv": np.ascontiguousarray(Wv[sl].T).astype(bf),
            "wo": np.ascontiguousarray(Wo[:, sl].T).astype(bf),
            "cosT": cosT,
            "sinT": sinT,
        })
    return in_maps


def kernel(x, Wq, Wk, Wv, Wo, _trace=False):
    if "nc" not in _CACHE:
        _CACHE["nc"] = _build_nc()
    nc = _CACHE["nc"]
    in_maps = _host_prep(x, Wq, Wk, Wv, Wo)
    res = run_bass_kernel_spmd(nc, in_maps, core_ids=list(range(8)), trace=_trace)
    _CACHE["last_result"] = res
    out = np.zeros((B, S, D_MODEL), dtype=np.float32)
    for core in range(8):
        out[core // 4] += res.results[core]["y"]
    return out
